# revision 5
# baseline (speedup 1.0000x reference)
"""V14: V13 + independent tail-half tiles (no WAW serialization).

V8 finding: all matmuls cost ~216ns (PE ingests 128 elem/cycle in any
dtype), so the PE lagged the now-smaller stream by ~3us, and the whole
scale+store tail (~4us) sat after the last byte.

1. fp8 chunk PAIRS use perf_mode=DoubleRow (supported for
   float8e4 x float8e4): lhsT = c28[:, k:k+2, :] ([128,2,16]),
   rhs = xt8[:, j:j+2, half] ([128,2,512]) -> effective K=256 at 0.5
   cycles/row: 4x less PE time for the fp8 stream.
2. Two PSUM accumulators: psfA closes 3 chunks before the end; its
   scale (vector+scalar halves) and store (sync+act queues) overlap the
   final taper chunks.  Only the last-3-chunk accumulator psfB's
   scale/store remains in the tail.  The host adds out_a + out_b
   (it already sums the 8 per-core partials).
3. Z chain sits right after the first group's matmuls (zpart is ready
   by then; reciprocal follows the c2 mult on Vector in program order).

Everything else as V8 (fp16/fp8 row routing at exp(w)>=THR, merged
meta DMA first on Sync, interleaved dual streams).
"""

import numpy as np
import ml_dtypes

import concourse.bass as bass
import concourse.tile as tile
from concourse import bacc, mybir
from concourse.bass_utils import run_bass_kernel_spmd
from concourse.vector_clock import ScopedClock


class _LeanTileContext(tile.TileContext):
    """TileContext with a minimal kernel epilogue.

    The stock epilogue attaches a wait on EVERY allocated semaphore's
    final value (the global tick clock) to the drain -- ~56 sems x 5
    engines of EVENT_SEMAPHORE wait-checks, ~7us of measured tail.  The
    per-engine drains + the all-engine barrier already guarantee each
    engine ran its last instruction and its DMA queues are empty (so all
    semaphore increments have landed) before the gpsimd range-clear."""

    def _drain_and_barrier(self, tick_clock, wait_clock):
        self.nc.sync.drain()
        self.nc.all_engine_barrier()
        popped = self.nc._tile_sem_poison_stack.pop()
        assert popped is self._sem_poison
        self.nc.clear_and_free_semaphores(list(self.sems.allocated().values()))

B, T, D = 16, 2048, 1024
NCORES = 8
F32 = mybir.dt.float32
F16 = mybir.dt.float16
F8 = mybir.dt.float8e4
NP_F8 = ml_dtypes.float8_e4m3

import os
GSZ16 = int(os.environ.get("KGSZ16", "4"))      # fp16 chunks per DMA group
GSZ8 = int(os.environ.get("KGSZ8", "6"))        # fp8 chunks per DMA group
WARMUP_MMS = int(os.environ.get("KWARM", "7"))
FILLER_MMS = int(os.environ.get("KFILL", "1"))
THR = float(os.environ.get("KTHR", "2.0"))      # exp(w) >= THR -> fp16
NTAIL = int(os.environ.get("KNTAIL", "3"))      # chunks on the B accumulator
BIG = 3.0e4           # t-sentinel (fp16-safe) for "not owned by this row"
WCOLS = T // 128      # 16


def _meta_layout(nchunks):
    off_w = 0
    off_wc = 16
    off_msk = ((off_wc + nchunks + 15) // 16) * 16
    G = off_msk + nchunks * B
    return off_w, off_wc, off_msk, G


def _chunk_groups(n, gsz, ntail):
    """Split n chunks into groups of ~gsz.  The first group is small
    (<=2 chunks) so the PE gets data as soon as the coefficients are
    ready; the last min(n, ntail) chunks are single-chunk groups."""
    singles = min(n, ntail)
    body = n - singles
    sizes = []
    rem = body
    if rem > 2:
        sizes.append(2); rem -= 2
    while rem > 0:
        s = min(gsz, rem); sizes.append(s); rem -= s
    sizes += [1] * singles
    groups = []
    k0 = 0
    for s in sizes:
        groups.append((k0, s)); k0 += s
    return groups


def _build_program(n16, n8):
    nc = bacc.Bacc(
        "TRN2", target_bir_lowering=False, debug=False, num_devices=NCORES
    )
    nchunks = n16 + n8
    ntail = 0
    off_w, off_wc, off_msk, G = _meta_layout(nchunks)
    xc16 = (nc.dram_tensor("xc16", [128, max(n16, 1), D], F16,
                           kind="ExternalInput").ap())
    xc8 = (nc.dram_tensor("xc8", [128, max(n8, 1), D], F8,
                          kind="ExternalInput").ap())
    meta = nc.dram_tensor("meta", [128, G], F16, kind="ExternalInput").ap()
    out = nc.dram_tensor("out", [B, D], F32, kind="ExternalOutput").ap()

    # merged group list: fp16 groups interleaved into the fp8 body by
    # byte progress; the fp8 taper singles close the stream.
    g16 = [("h", k0, gs) for k0, gs in _chunk_groups(n16, GSZ16, 0)]
    g8 = [("l", k0, gs) for k0, gs in _chunk_groups(n8, GSZ8, min(ntail, n8))]
    bytes16 = 2 * D * 128
    bytes8 = 1 * D * 128
    tot16 = sum(gs for _, _, gs in g16) * bytes16
    tot8 = sum(gs for _, _, gs in g8) * bytes8
    ntaper = min(ntail, n8)
    merged = []
    i = j = 0
    acc16 = acc8 = 0
    while i < len(g16) or j < len(g8):
        if i < len(g16) and (
            j >= len(g8) - ntaper
            or (tot8 and acc8 / tot8 >= acc16 / max(tot16, 1))
        ):
            merged.append(g16[i]); acc16 += g16[i][2] * bytes16; i += 1
        elif j < len(g8):
            merged.append(g8[j]); acc8 += g8[j][2] * bytes8; j += 1
    ng = len(merged)

    with _LeanTileContext(nc) as tc:
        with (
            tc.tile_pool(name="consts", bufs=1) as consts,
            tc.tile_pool(name="xin", bufs=1) as xpool,
            tc.tile_pool(name="outs", bufs=1) as opool,
            tc.tile_pool(name="psum", bufs=1, space="PSUM") as pacc,
            tc.tile_pool(name="psumz", bufs=1, space="PSUM") as pz,
        ):
            # --- ONE metadata load, first on the Sync HWDGE queues ---
            meta_sb = consts.tile([128, G], F16)
            nc.sync.dma_start(out=meta_sb, in_=meta)

            def view(off, dims):
                return bass.AP(
                    tensor=meta_sb.tensor, offset=meta_sb.offset + off,
                    ap=[meta_sb.ap[0]] + dims,
                )

            w2_v = view(off_w, [[1, WCOLS]])
            wc_v = view(off_wc, [[1, nchunks]])
            msk_v = view(off_msk, [[B, nchunks], [1, B]])

            # --- start the X stream ---
            from collections import Counter
            tag_counts = Counter((st, gs) for st, _, gs in merged)

            def x_dma(st, k0, gs):
                if st == "h":
                    xt = xpool.tile([128, gs, D], F16, name="xt",
                                    tag=f"xh{gs}", bufs=tag_counts[(st, gs)])
                    nc.sync.dma_start(out=xt, in_=xc16[:, k0 : k0 + gs])
                else:
                    xt = xpool.tile([128, gs, D], F8, name="xt",
                                    tag=f"xl{gs}", bufs=tag_counts[(st, gs)])
                    nc.sync.dma_start(out=xt, in_=xc8[:, k0 : k0 + gs])
                return xt

            xts = []
            for st, k0, gs in merged[:2]:
                xts.append(x_dma(st, k0, gs))

            # --- coefficient pipeline (host ships the 0/1 ownership
            # mask, so c2 is a single multiply and c28 is independent) ---
            ec = consts.tile([128, nchunks], F16)
            nc.scalar.activation(
                out=ec, in_=wc_v, func=mybir.ActivationFunctionType.Exp,
            )
            ec_b = bass.AP(
                tensor=ec.tensor, offset=ec.offset,
                ap=[ec.ap[0], ec.ap[1], [0, B]],
            )
            c2 = consts.tile([128, nchunks, B], F16)
            nc.vector.tensor_tensor(
                out=c2, in0=msk_v, in1=ec_b, op=mybir.AluOpType.mult,
            )
            if n8:
                c28 = consts.tile([128, n8, B], F8)
                msk_lo = view(off_msk + n16 * B, [[B, n8], [1, B]])
                ec_lo_b = bass.AP(
                    tensor=ec.tensor, offset=ec.offset + n16,
                    ap=[ec.ap[0], [1, n8], [0, B]],
                )
                nc.vector.tensor_tensor(
                    out=c28, in0=msk_lo, in1=ec_lo_b, op=mybir.AluOpType.mult,
                )
            # exp over all T weights, per-partition partial sums
            e2 = consts.tile([128, WCOLS], F32)
            zpart = consts.tile([128, 1], F32)
            nc.scalar.activation(
                out=e2, in_=w2_v, func=mybir.ActivationFunctionType.Exp,
                accum_out=zpart,
            )

            # --- PE warm-up (keeps the HAM clock gate at full speed) ---
            warm_rhs = consts.tile([128, 512], F16)
            nc.vector.memset(warm_rhs.bitcast(F32), 0.0)
            warm_lhs = consts.tile([128, 16], F16)
            nc.vector.memset(warm_lhs.bitcast(F32), 0.0)
            ones128 = consts.tile([128, B], F32)
            nc.vector.memset(ones128, 1.0)
            pwarm = pz.tile([16, 512], F32)
            for _ in range(WARMUP_MMS):
                nc.tensor.matmul(pwarm, lhsT=warm_lhs, rhs=warm_rhs,
                                 start=True, stop=True)

            # --- main streaming loop, two accumulators ---
            psfA = pacc.tile([B, D], F32, name="psfA", tag="psA")
            na = nchunks
            rz = consts.tile([B, 1], F32)
            rz2 = consts.tile([B, 1], F32)
            psum_z = pz.tile([B, 1], F32)
            ot = opool.tile([B, D], F32)

            def mm(psf, lhs, rhs_, start, stop, perf_mode=None):
                nc.tensor.matmul(
                    psf, lhsT=lhs, rhs=rhs_, start=start, stop=stop,
                    perf_mode=perf_mode,
                )

            seen = 0
            for g, (st, k0, gs) in enumerate(merged):
                xt = xts[g] if g < 2 else x_dma(st, k0, gs)
                j = 0
                while j < gs:
                    a_side = True
                    psf = psfA
                    lim = na - seen
                    first = seen == 0
                    # fp8 pairs via DoubleRow when both chunks land on the
                    # same accumulator
                    if st == "l" and j + 1 < gs and lim >= 2:
                        for dh in range(2):
                            mm(psf[:, dh * 512 : (dh + 1) * 512],
                               c28[:, k0 + j : k0 + j + 2, :],
                               xt[:, j : j + 2, dh * 512 : (dh + 1) * 512],
                               start=first, stop=(seen + 2 == na if a_side
                                                  else seen + 2 == nchunks),
                               perf_mode=mybir.MatmulPerfMode.DoubleRow)
                        seen += 2; j += 2
                    else:
                        if st == "h":
                            lhs = c2[:, k0 + j, :]
                        else:
                            lhs = c28[:, k0 + j, :]
                        for dh in range(2):
                            mm(psf[:, dh * 512 : (dh + 1) * 512], lhs,
                               xt[:, j, dh * 512 : (dh + 1) * 512],
                               start=first, stop=(seen + 1 == na if a_side
                                                  else seen + 1 == nchunks))
                        seen += 1; j += 1
                if g == 0:
                    # Z broadcast + reciprocal while the stream runs
                    nc.tensor.matmul(psum_z, lhsT=ones128, rhs=zpart,
                                     start=True, stop=True)
                    nc.vector.reciprocal(rz, psum_z)
                    # scalar-owned copy: the tail COPY then waits on the
                    # scalar clock, not on later vector-engine ticks
                    nc.scalar.activation(
                        out=rz2, in_=rz,
                        func=mybir.ActivationFunctionType.Copy,
                    )
                if g < ng - 1:
                    for _ in range(FILLER_MMS):
                        nc.tensor.matmul(pwarm, lhsT=warm_lhs, rhs=warm_rhs,
                                         start=True, stop=True)

            nc.vector.tensor_scalar(
                out=ot[:, 0:512], in0=psfA[:, 0:512], scalar1=rz,
                scalar2=None, op0=mybir.AluOpType.mult,
            )
            nc.sync.dma_start(out=out[:, 0:512], in_=ot[:, 0:512])
            nc.scalar.activation(
                out=ot[:, 512:1024], in_=psfA[:, 512:1024],
                func=mybir.ActivationFunctionType.Copy, scale=rz,
            )
            nc.scalar.dma_start(out=out[:, 512:1024], in_=ot[:, 512:1024])

    nc.compile()
    return nc


_cache = {}


def _get_program(key):
    if key not in _cache:
        _cache[key] = _build_program(*key)
    return _cache[key]


def _pack_rows(b_all, t_all, ncols):
    cap = NCORES * ncols * 128
    pad = cap - len(b_all)
    b = np.concatenate([b_all, np.full(pad, -1, dtype=np.int64)])
    t = np.concatenate([t_all, np.zeros(pad, dtype=np.int64)])
    return b.reshape(NCORES, ncols, 128), t.reshape(NCORES, ncols, 128)


def kernel(input, lengths, weights):
    input = np.asarray(input, dtype=np.float32)
    lengths_np = np.asarray(lengths).astype(np.int64)
    weights = np.asarray(weights, dtype=np.float32)

    lens_clip = np.clip(lengths_np, 0, T)
    total_rows = int(lens_clip.sum())

    b_flat = np.repeat(np.arange(B, dtype=np.int64), lens_clip)
    t_flat = np.concatenate(
        [np.arange(n, dtype=np.int64) for n in lens_clip]
    ) if total_rows else np.zeros(0, dtype=np.int64)

    # route rows by softmax weight: exp(w_t) >= THR -> fp16 stream
    ew = np.exp(weights)
    hi_mask = ew[t_flat] >= THR if total_rows else np.zeros(0, dtype=bool)
    bh, th = b_flat[hi_mask], t_flat[hi_mask]
    bl, tl = b_flat[~hi_mask], t_flat[~hi_mask]

    n16 = -(-len(bh) // (NCORES * 128)) if len(bh) else 0
    n8 = -(-len(bl) // (NCORES * 128)) if len(bl) else 0
    if n16 == 0 and n8 == 0:
        n8 = 1
    nchunks = n16 + n8

    bs16, ts16 = _pack_rows(bh, th, n16) if n16 else (
        np.full((NCORES, 0, 128), -1, np.int64),
        np.zeros((NCORES, 0, 128), np.int64))
    bs8, ts8 = _pack_rows(bl, tl, n8) if n8 else (
        np.full((NCORES, 0, 128), -1, np.int64),
        np.zeros((NCORES, 0, 128), np.int64))

    nc = _get_program((n16, n8))
    off_w, off_wc, off_msk, G = _meta_layout(nchunks)

    w2_np = weights.reshape(128, WCOLS).astype(np.float16)

    flat16 = input.reshape(B * T, D).astype(np.float16)
    flat8 = input.reshape(B * T, D).astype(NP_F8)
    rb = np.arange(B)
    in_maps = []
    for c in range(NCORES):
        bs = np.concatenate([bs16[c], bs8[c]], axis=0)   # [nchunks, 128]
        ts = np.concatenate([ts16[c], ts8[c]], axis=0)

        xc16 = flat16[np.maximum(bs16[c], 0) * T + ts16[c]].transpose(1, 0, 2)
        xc8 = flat8[np.maximum(bs8[c], 0) * T + ts8[c]].transpose(1, 0, 2)
        if n16 == 0:
            xc16 = np.zeros((128, 1, D), np.float16)
        if n8 == 0:
            xc8 = np.zeros((128, 1, D), NP_F8)
        wc = weights[ts].T.astype(np.float16)            # [128, nchunks]
        # 0/1 ownership mask: row (chunk, p) belongs to batch b and is live
        msk = (
            (bs[:, :, None] == rb[None, None, :])
            & (ts[:, :, None] < lens_clip[None, None, :])
        ).transpose(1, 0, 2).astype(np.float16)          # [128, nchunks, B]

        meta = np.zeros((128, G), dtype=np.float16)
        meta[:, off_w : off_w + WCOLS] = w2_np
        meta[:, off_wc : off_wc + nchunks] = wc
        meta[:, off_msk : off_msk + nchunks * B] = msk.reshape(128, -1)

        in_maps.append(
            {
                "xc16": np.ascontiguousarray(xc16),
                "xc8": np.ascontiguousarray(xc8),
                "meta": meta,
            }
        )

    res = run_bass_kernel_spmd(nc, in_maps, list(range(NCORES)))
    out = np.zeros((B, D), dtype=np.float32)
    for c in range(NCORES):
        out += res.results[c]["out"]
    return out.astype(np.float32)
